# revision 7
# baseline (speedup 1.0000x reference)
"""ALiBi bidirectional attention — 8-core Trainium2 Bass kernel (v2).

Problem: B=2, T=2048, D=1024, H=16, hd=64, f32 in/out.
reference: softmax(Q K^T/8 + slopes_h * -|i-j|) V, then out-proj.

Sharding (sequence-parallel): core c handles batch c//4 and query rows
q0 = 512*(c%4) .. q0+512. Out-proj contracts the full model dim locally,
so the output is a pure concat of per-core [512, 1024] slices.

K^T and V are projected on the owning slice and AllGathered within the
4-core batch group (two 1MB-in bf16 collectives). SPMD rotation:
k-position data lives in per-core LOCAL coordinates
k_local = (k_phys - q0) mod 2048; gather-back DMAs use a host-passed
block-rotation table via register-offset APs, so the graph is identical
on every core.

ALiBi: with s = bf16-snapped slope and diff = k_phys - q_phys:
  * non-crossing k-tiles (local kt 4..15): bias is affine per tile; the
    exp's [P,1] bias carries -+s*p; two extra contract rows in the
    scores matmul carry the q_lo part and the per-tile constant.
  * crossing tiles (local kt 0..3): scores exp'd raw then multiplied by
    exp(-s|diff|) from a host-precomputed shifted-window table
    EW[p,h,col]=exp(-s_h|p-col+384|).
Scores are computed transposed (ST = [kpos, q]) so probs feed the AV
matmul as lhsT-ready; a ones column in V yields softmax row-sums in the
same matmul; no row-max pass (args <= ~6, exp cannot overflow).
(h, kt-pair) groups with s_h*min|diff| >= 5 are skipped (dropped
attention mass adds ~1e-4 rel err vs the 5.9e-3 bf16 noise floor).

Own-block scores use PE row-tiling (64x128 mode): even k-tiles at SBUF
partitions 0-63 (tile T0), odd at partitions 64-127 (T8), so two
contract-64 matmuls run concurrently in the array.

Schedule (per core): K-proj -> AllGather K; V-proj (own 4 tiles) ->
AllGather V -> rotated gather-back DMAs; Q-proj; then ONE unified
per-head pipeline: [own scores (paired) + rest scores] -> exps ->
all AVs accumulating one [65,QS] psum -> 1/z normalize; out-proj.
"""
import math
import sys

sys.path.insert(0, "/opt/trn_rl_repo")

import numpy as np

from concourse import bass, bacc
import concourse.tile as tile
from concourse.bass_utils import run_bass_kernel_spmd

mybir = bass.mybir
FP32 = mybir.dt.float32
BF16 = mybir.dt.bfloat16
INT32 = mybir.dt.int32

B, T, D = 2, 2048, 1024
H, HD = 16, 64
NCORES = 8
QS = 512                      # query rows per core
NKT = T // 128                # 16 k tiles
GROUPS = [[0, 1, 2, 3], [4, 5, 6, 7]]

try:
    import ml_dtypes
    BF16_NP = np.dtype(ml_dtypes.bfloat16)
except ImportError:
    BF16_NP = None


def _bf16_round_f32(x):
    u = np.asarray(x, np.float32).view(np.uint32)
    r = (u + 0x7FFF + ((u >> 16) & 1)) & 0xFFFF0000
    return r.astype(np.uint32).view(np.float32)


def _slopes():
    start = 2.0 ** (-(2.0 ** (-(math.log2(H) - 3))))
    return np.asarray([start * start ** i for i in range(H)], np.float32)


SLOPES = _bf16_round_f32(_slopes())     # used consistently everywhere

SKIP_THRESH = 5.0


def _skippable(h, kt):
    m = min(128 * kt - 511, 1921 - 128 * kt)
    return SLOPES[h] * m >= SKIP_THRESH


SKIP_GROUPS = [
    {g for g in range(6)
     if _skippable(h, 4 + 2 * g) and _skippable(h, 5 + 2 * g)}
    for h in range(H)
]
# most-work heads first so the pipeline tail drains on cheap heads
HEAD_ORDER = sorted(range(H), key=lambda h: len(SKIP_GROUPS[h]))

# --------------------------------------------------------------------------
# graph
# --------------------------------------------------------------------------


def _build_graph():
    nc = bacc.Bacc("TRN2", target_bir_lowering=False, debug=False,
                   num_devices=NCORES)

    p = {}
    p["xq"] = nc.declare_dram_parameter("xq", [D, QS], BF16, isOutput=False)
    p["xk"] = nc.declare_dram_parameter("xk", [D, QS], BF16, isOutput=False)
    p["xv"] = nc.declare_dram_parameter("xv", [D, QS], BF16, isOutput=False)
    for nm in ("wq", "wk", "wv", "wo"):
        p[nm] = nc.declare_dram_parameter(nm, [D, D], BF16, isOutput=False)
    p["qlo"] = nc.declare_dram_parameter("qlo", [2, H, QS], BF16, isOutput=False)
    p["srow"] = nc.declare_dram_parameter("srow", [H, 2, T], BF16, isOutput=False)
    p["biasall"] = nc.declare_dram_parameter("biasall", [128, H * 8], FP32,
                                             isOutput=False)
    p["ewt"] = nc.declare_dram_parameter("ewt", [128, H * 896], BF16,
                                         isOutput=False)
    p["rotidx"] = nc.declare_dram_parameter("rotidx", [1, 4], INT32,
                                            isOutput=False)
    p["out"] = nc.declare_dram_parameter("out", [QS, D], FP32, isOutput=True)

    bounce_k = nc.dram_tensor("bounce_k", [D, QS], BF16)
    agk = nc.dram_tensor("agk", [4, D, QS], BF16)
    bounce_v = nc.dram_tensor("bounce_v", [QS, D], BF16)
    agv = nc.dram_tensor("agv", [4, QS, D], BF16)

    with tile.TileContext(nc) as tc:
        _emit(tc, nc, p, bounce_k, agk, bounce_v, agv)

    nc.compile()
    return nc


def _emit(tc, nc, p, bounce_k, agk, bounce_v, agv):
    Exp = mybir.ActivationFunctionType.Exp
    import contextlib
    ctx = contextlib.ExitStack()

    cpool = ctx.enter_context(tc.tile_pool(name="consts", bufs=1))
    kvq = ctx.enter_context(tc.tile_pool(name="kvq", bufs=1))
    late = ctx.enter_context(tc.tile_pool(name="late", bufs=1))

    rot_sb = cpool.tile([1, 4], INT32)
    nc.gpsimd.dma_start(rot_sb[:], p["rotidx"].ap())
    qt = kvq.tile([66, H, QS], BF16)            # Q^T (+qlo rows, rest pass)
    qt2u = kvq.tile([128, H, QS], BF16)         # Q^T copy at partitions 64+
    klocal2 = kvq.tile([128, H, 256], BF16)     # own K^T paired even|odd
    vfull = kvq.tile([128, NKT, H, 65], BF16)   # full V, local coords (+ones)
    ew = cpool.tile([128, H, 896], BF16)        # crossing-tile exp windows
    biasall = cpool.tile([128, H * 8], FP32)

    def cast(idx, dst, src):
        # alternate psum->sbuf casts across the two free engines
        if idx % 2 == 0:
            nc.scalar.copy(dst, src)
        else:
            nc.vector.tensor_copy(dst, src)

    # ================= phase 1: projections + collectives =================
    pctx = contextlib.ExitStack()
    xw = pctx.enter_context(tc.tile_pool(name="xw", bufs=2))
    wrot = pctx.enter_context(tc.tile_pool(name="wrot", bufs=2))
    kl_pool = pctx.enter_context(tc.tile_pool(name="klp", bufs=1))

    def load_split(xt, wt, xnm, wnm, eng):
        # chunked input DMAs (cj0 first) so compute starts on chunk 0
        xsrc = p[xnm].ap().rearrange("(j p) c -> p j c", p=128)
        wsrc = p[wnm].ap().rearrange("(j p) c -> p j c", p=128)
        eng.dma_start(xt[:, 0, :], xsrc[:, 0, :])
        eng.dma_start(wt[:, 0, :], wsrc[:, 0, :])
        eng.dma_start(xt[:, 1:4, :], xsrc[:, 1:4, :])
        eng.dma_start(wt[:, 1:4, :], wsrc[:, 1:4, :])
        eng.dma_start(xt[:, 4:8, :], xsrc[:, 4:8, :])
        eng.dma_start(wt[:, 4:8, :], wsrc[:, 4:8, :])

    with tc.tile_pool(name="pp8", bufs=1, space="PSUM") as pp8:
        # ---- K projection (own slice), cj-outer over 8 live psum banks ---
        xk = xw.tile([128, 8, QS], BF16, tag="xk")
        wk_sb = wrot.tile([128, 8, D], BF16, tag="w")
        load_split(xk, wk_sb, "xk", "wk", nc.sync)
        # early big loads on the other queues while sync streams K inputs
        xv = xw.tile([128, 8, QS], BF16, tag="xv")
        wv_sb = wrot.tile([128, 8, D], BF16, tag="w")
        load_split(xv, wv_sb, "xv", "wv", nc.scalar)
        nc.gpsimd.dma_start(ew[:], p["ewt"].ap().rearrange(
            "p (h c) -> p h c", h=H))
        nc.gpsimd.dma_start(biasall[:], p["biasall"].ap())
        nc.gpsimd.dma_start(qt[64:66, :, :], p["qlo"].ap())

        klocal = kl_pool.tile([64, H, QS], BF16)
        psk = [pp8.tile([128, QS], FP32, tag=f"p{j}", name=f"psk{j}")
               for j in range(8)]
        for cj in range(8):
            for j in range(8):
                nc.tensor.matmul(psk[j][:], wk_sb[:, cj, 128 * j:128 * (j + 1)],
                                 xk[:, cj, :], start=(cj == 0), stop=(cj == 7))
        for j in range(8):
            cast(0, klocal[:, 2 * j, :], psk[j][0:64, :])
            cast(1, klocal[:, 2 * j + 1, :], psk[j][64:128, :])
        nc.sync.dma_start(
            bounce_k.ap().rearrange("(h p) c -> p h c", p=64), klocal[:])
        nc.gpsimd.collective_compute(
            "AllGather", mybir.AluOpType.bypass, replica_groups=GROUPS,
            ins=[bounce_k.ap().opt()], outs=[agk.ap().opt()])
        # own K^T pair layout: even tiles -> partitions 0-63, odd -> 64-127
        for g in range(2):
            nc.sync.dma_start(klocal2[0:64, :, 128 * g:128 * (g + 1)],
                              klocal[:, :, 256 * g:256 * g + 128])
            nc.sync.dma_start(klocal2[64:128, :, 128 * g:128 * (g + 1)],
                              klocal[:, :, 256 * g + 128:256 * g + 256])

        # ---- V projection (own 4 k-tiles), cj-outer over 8 psum banks ----
        nc.vector.memset(vfull[:, :, :, 64:65], 1.0)
        psv = [pp8.tile([128, QS], FP32, tag=f"p{j}", name=f"psv{j}")
               for j in range(8)]
        for cj in range(8):
            for tc_i in range(4):
                for nh in range(2):
                    nc.tensor.matmul(psv[2 * tc_i + nh][:],
                                     xv[:, cj, 128 * tc_i:128 * (tc_i + 1)],
                                     wv_sb[:, cj, 512 * nh:512 * (nh + 1)],
                                     start=(cj == 0), stop=(cj == 7))
        for tc_i in range(4):
            for nh in range(2):
                cast(nh, vfull[:, tc_i, 8 * nh:8 * (nh + 1), 0:64],
                     psv[2 * tc_i + nh][:].rearrange("p (h d) -> p h d", h=8))
        bv = bounce_v.ap().rearrange("(t p) (h d) -> p t h d", p=128, h=H)
        for t in range(4):
            nc.sync.dma_start(bv[:, t, :, :], vfull[:, t, :, 0:64])
        nc.gpsimd.collective_compute(
            "AllGather", mybir.AluOpType.bypass, replica_groups=GROUPS,
            ins=[bounce_v.ap().opt()], outs=[agv.ap().opt()])

        # ---- Q projection ----
        xq = xw.tile([128, 8, QS], BF16, tag="xk")
        wq_sb = wrot.tile([128, 8, D], BF16, tag="w")
        load_split(xq, wq_sb, "xq", "wq", nc.scalar)
        psq = [pp8.tile([128, QS], FP32, tag=f"p{j}", name=f"psq{j}")
               for j in range(8)]
        for cj in range(8):
            for j in range(8):
                nc.tensor.matmul(psq[j][:], wq_sb[:, cj, 128 * j:128 * (j + 1)],
                                 xq[:, cj, :], start=(cj == 0), stop=(cj == 7))
        for j in range(8):
            cast(0, qt[0:64, 2 * j, :], psq[j][0:64, :])
            cast(1, qt[0:64, 2 * j + 1, :], psq[j][64:128, :])
        nc.sync.dma_start(qt2u[64:128, :, :], qt[0:64, :, :])

    # --- per-core K/V rotation registers ---
    rvs = []
    for rl in range(4):
        reg = nc.sync.alloc_register(f"rot{rl}")
        nc.sync.reg_load(reg, rot_sb[0:1, rl:rl + 1])
        rvs.append(nc.sync.snap(reg, donate=True))
    agk_r = agk.ap().rearrange("r (h d) c -> r d h c", h=H)    # [4,64,H,QS]
    agv_r = agv.ap().rearrange("r (t p) (h d) -> r p t h d", p=128, h=H)

    # gathered V -> vfull local tiles 4..15 (block rl holds local 4rl..4rl+3)
    for rl in range(1, 4):
        for t in range(4):
            nc.sync.dma_start(vfull[:, 4 * rl + t, :, 0:64],
                              agv_r[bass.ds(rvs[rl], 1), :, t, :, :])

    pctx.close()   # xw/wrot/klocal SBUF freed for the attention pools

    # ================= phase 2: unified per-head attention ================
    wo_sb = late.tile([128, 8, D], BF16)
    nc.gpsimd.dma_start(wo_sb[:], p["wo"].ap().rearrange(
        "(j p) c -> p j c", p=128))
    with tc.tile_pool(name="otn", bufs=1) as otpool, \
         tc.tile_pool(name="ktstream", bufs=4) as kts, \
         tc.tile_pool(name="exps", bufs=4) as epool, \
         tc.tile_pool(name="recip", bufs=3) as rpool, \
         tc.tile_pool(name="yout", bufs=2) as ypool, \
         tc.tile_pool(name="stps", bufs=3, space="PSUM") as stp, \
         tc.tile_pool(name="otps", bufs=2, space="PSUM") as otp:

        ot = otpool.tile([128, 8, QS], BF16)    # normalized O^T
        for hi, h in enumerate(HEAD_ORDER):
            kept = [g for g in range(6) if g not in SKIP_GROUPS[h]]
            kth = kts.tile([66, 3 * QS], BF16, tag="kth")
            for rl in range(1, 4):
                if all((kt - 4) // 2 in SKIP_GROUPS[h]
                       for kt in range(4 * rl, 4 * rl + 4)):
                    continue    # whole block below the mass threshold
                nc.sync.dma_start(kth[0:64, QS * (rl - 1):QS * rl],
                                  agk_r[bass.ds(rvs[rl], 1), :, h, :])
            nc.sync.dma_start(kth[64:66, :], p["srow"].ap()[h, :, QS:])

            # ---- scores: own pairs (row-tiled) then kept rest groups ----
            own_sts = []
            for g in range(2):
                stps = stp.tile([128, 2 * QS], FP32, tag="st")
                nc.tensor.matmul(stps[:, 0:QS],
                                 klocal2[0:64, h, 128 * g:128 * (g + 1)],
                                 qt[0:64, h, :], start=True, stop=True)
                nc.tensor.matmul(stps[:, QS:2 * QS],
                                 klocal2[64:128, h, 128 * g:128 * (g + 1)],
                                 qt2u[64:128, h, :], start=True, stop=True)
                own_sts.append(stps)

            otps = otp.tile([65, QS], FP32, tag="ot")
            nmm = 4 + 2 * len(kept)
            mi = 0

            def av(kt, e_half):
                nonlocal mi
                nc.tensor.matmul(otps[:], vfull[:, kt, h, :], e_half,
                                 start=(mi == 0), stop=(mi == nmm - 1))
                mi += 1

            # own-block: exp raw scores, EW-window multiply, AV kt 0..3
            for g in range(2):
                e = epool.tile([128, 2 * QS], BF16, tag="e")
                nc.scalar.activation(e[:], own_sts[g][:], Exp,
                                     bias=biasall[:, 8 * h:8 * h + 1],
                                     scale=1.0)
                for j in range(2):
                    kt = 2 * g + j
                    nc.vector.tensor_mul(e[:, QS * j:QS * (j + 1)],
                                         e[:, QS * j:QS * (j + 1)],
                                         ew[:, h, 384 - 128 * kt:896 - 128 * kt])
                    av(kt, e[:, QS * j:QS * (j + 1)])

            # rest groups: scores (contract 66) -> exp (affine bias) -> AV
            for g in kept:
                stps = stp.tile([128, 2 * QS], FP32, tag="st")
                for j in range(2):
                    kt = 4 + 2 * g + j
                    nc.tensor.matmul(stps[:, QS * j:QS * (j + 1)],
                                     kth[:, 128 * (kt - 4):128 * (kt - 3)],
                                     qt[:, h, :], start=True, stop=True)
                e = epool.tile([128, 2 * QS], BF16, tag="e")
                nc.scalar.activation(e[:], stps[:], Exp,
                                     bias=biasall[:, 8 * h + 1 + g:8 * h + 2 + g],
                                     scale=1.0)
                for j in range(2):
                    av(4 + 2 * g + j, e[:, QS * j:QS * (j + 1)])

            # ---- normalize: 1/z broadcast, pack 2 heads per column block --
            zrow = rpool.tile([1, QS], FP32, tag="zrow")
            nc.vector.tensor_copy(zrow[:], otps[64:65, :])
            rec = rpool.tile([1, QS], FP32, tag="rec")
            # approx recip needs a partition-0 fp32 source; ~51 ULP is plenty
            nc.vector.reciprocal_approx_fast(rec[:], zrow[:])
            bcs = rpool.tile([64, QS], FP32, tag="bcs")
            nc.gpsimd.partition_broadcast(bcs[:], rec[:])
            nc.vector.tensor_mul(ot[64 * (h % 2):64 * (h % 2) + 64, h // 2, :],
                                 otps[0:64, :], bcs[:])

        # --- out-projection (reuses the "ot" psum slots) ---
        for tc_i in range(4):
            y = ypool.tile([128, D], FP32, tag="y")
            for nh in range(2):
                ps = otp.tile([128, 512], FP32, tag="ot", name=f"ops{tc_i}{nh}")
                for j in range(8):
                    nc.tensor.matmul(ps[:], ot[:, j, 128 * tc_i:128 * (tc_i + 1)],
                                     wo_sb[:, j, 512 * nh:512 * (nh + 1)],
                                     start=(j == 0), stop=(j == 7))
                cast(nh, y[:, 512 * nh:512 * (nh + 1)], ps[:])
            nc.sync.dma_start(p["out"].ap()[128 * tc_i:128 * (tc_i + 1), :], y[:])

    ctx.close()


# --------------------------------------------------------------------------
# host side
# --------------------------------------------------------------------------

_EWT_CACHE = {}


def _ewt():
    if "ewt" not in _EWT_CACHE:
        pvec = np.arange(128, dtype=np.float32)
        col = np.arange(896, dtype=np.float32)
        base = np.abs(pvec[:, None] - col[None, :] + 384.0)
        ewt = np.empty((128, H, 896), np.float32)
        for h in range(H):
            ewt[:, h, :] = np.exp(-SLOPES[h] * base)
        _EWT_CACHE["ewt"] = ewt.reshape(128, H * 896).astype(BF16_NP)
    return _EWT_CACHE["ewt"]


def _prep_core_inputs(inputs, c):
    b, s = divmod(c, 4)
    q0 = QS * s
    sl = slice(q0, q0 + QS)
    f32 = np.float32

    for bn in ("bq", "bk", "bv", "bo"):
        assert not np.any(np.asarray(inputs[bn])), \
            f"nonzero {bn} not supported by this kernel build"

    def tr(x):
        return np.ascontiguousarray(np.asarray(x, f32).T)

    m = {
        "xq": tr(inputs["query"][b][sl]).astype(BF16_NP),
        "xk": tr(inputs["key"][b][sl]).astype(BF16_NP),
        "xv": tr(inputs["value"][b][sl]).astype(BF16_NP),
        "wq": (np.asarray(inputs["Wq"], f32) * HD ** -0.5).astype(BF16_NP),
        "wk": np.asarray(inputs["Wk"], f32).astype(BF16_NP),
        "wv": np.asarray(inputs["Wv"], f32).astype(BF16_NP),
        "wo": np.asarray(inputs["Wo"], f32).astype(BF16_NP),
        "ewt": _ewt(),
    }

    qlo = np.zeros((2, H, QS), f32)
    qlo[0] = (np.arange(QS, dtype=f32) - 256.0)[None, :]
    qlo[1] = (128.0 * SLOPES)[:, None]
    m["qlo"] = qlo.astype(BF16_NP)

    # local k coords; wrap where k_local >= T - q0 (512-aligned)
    kloc = np.arange(T)
    wrap = kloc >= (T - q0) if q0 > 0 else np.zeros(T, bool)
    ktv = kloc // 128
    srow = np.zeros((H, 2, T), f32)
    biasall = np.zeros((128, H, 8), f32)
    pvec = np.arange(128, dtype=f32)
    for h in range(H):
        sh = SLOPES[h]
        # row 0: coefficient of (q_lo - 256); row 1: coefficient of 128*s
        srow[h, 0, 512:] = np.where(wrap[512:], -sh, sh)
        srow[h, 1, 512:] = np.where(wrap[512:], ktv[512:] - 18.0,
                                    2.0 - ktv[512:])
        for g in range(6):
            kt = 4 + 2 * g
            biasall[:, h, 1 + g] = (sh * pvec) if wrap[128 * kt] else (-sh * pvec)
    m["srow"] = srow.astype(BF16_NP)
    m["biasall"] = biasall.reshape(128, H * 8)

    m["rotidx"] = np.asarray([[(rl + s) % 4 for rl in range(4)]], np.int32)
    return m


_NC_CACHE = {}


def _get_nc():
    if "nc" not in _NC_CACHE:
        _NC_CACHE["nc"] = _build_graph()
    return _NC_CACHE["nc"]


def run(inputs, trace=False, trace_kwargs=None):
    nc = _get_nc()
    in_maps = [_prep_core_inputs(inputs, c) for c in range(NCORES)]
    res = run_bass_kernel_spmd(nc, in_maps, list(range(NCORES)),
                               trace=trace, **(trace_kwargs or {}))
    out = np.empty((B, T, D), np.float32)
    for c in range(NCORES):
        b, s = divmod(c, 4)
        out[b, QS * s:QS * (s + 1), :] = res.results[c]["out"]
    return out, res


def kernel(**inputs):
    return run(inputs)[0]
